# revision 9
# baseline (speedup 1.0000x reference)
"""Trainium2 Bass kernel for cross-attention (b=2, n=m=2048, dim=1024, 16 heads x 64)
with QK-RMSNorm and rotate-half RoPE (float positions), distributed over 8 NeuronCores.

Sharding: core c handles batch b = c//4 and head group hg = c%4 (4 heads each).
Wq/Wkv are column-sharded by head, Wo row-sharded; each core emits a partial
[2048, 1024] output which the host sums over the 4 cores of each batch (the
row-parallel all-reduce done at unshard time).

Device dataflow per core (all matmuls bf16, softmax f32 on ScalarE):
  tgt^T/src^T (host-transposed f32) --gpsimd DMA cast--> bf16 [128, 8, 2048]
  KV = src @ Wkv        (PE, natural layout [t, 512])
  Q  = tgt @ Wq         (PE, natural layout [t, 256])
  RMS stats (DVE square+reduce) -> rsq = exp(-0.5*ln(ms+eps)) (ACT, ln/exp set)
  normalize+cast (DVE), RoPE via w-folded sin/cos tables (DVE TT ops; trig from
  ACT Sin with Cody-Waite range reduction; cos = sin(x+pi/2))
  Q_hat/K_hat --DRAM bounce--> DMA-xbar transpose --> [hd, t] layout
  S^T = K_hat^T.T @ Q_hat^T   (row-tiled head pairs, K_c=64, full PE rate)
  P^T = exp(S^T)              (ACT, psum->sbuf bf16, [128, 1024] ops)
  O^T/denom = [V|1].T @ P^T   (PE, M=65: AV and softmax denominator together)
  normalize O^T (DVE, reciprocal + DMA partition-broadcast)
  out = O^T.T @ Wo            (PE, accumulate over c-chunks, natural layout)
"""

import math
import os

import numpy as np

B, N, DIM, H, HD = 2, 2048, 1024, 16, 64
NCORES = 8
HPC = 4  # heads per core
QD = HPC * HD  # 256
P = 128
NT = N // P  # 16 token chunks
KC = DIM // P  # 8 contraction chunks
QB = 4  # q blocks of 512
QW = N // QB  # 512
ROPE_THETA = 10000.0
EPS = float(np.finfo(np.float32).eps)

_CACHE = {}
LAST_RESULTS = None


def _build_trig(nc, tc, pool, scr, pos_dram, w_sb, invf_sb, consts, tag):
    """sin/cos tables with RMSNorm-weight w folded in. Returns (cw1, cw2, sw1, sw2),
    each [P, NT, 32] bf16: cw1=cos*w[0:32], cw2=cos*w[32:64], sw1=sin*w[0:32],
    sw2=sin*w[32:64]."""
    import concourse.bass as bass
    from concourse import mybir

    f32 = mybir.dt.float32
    bf16 = mybir.dt.bfloat16
    AF = mybir.ActivationFunctionType
    ALU = mybir.AluOpType
    INV2PI, MAGIC, C1, C2, C3 = consts

    pos_sb = scr.tile([P, NT], f32, tag=f"pos{tag}")
    nc.sync.dma_start(out=pos_sb, in_=pos_dram.rearrange("(c p) -> p c", p=P))
    ang = scr.tile([P, NT * 32], f32, tag=f"ang{tag}")
    nc.vector.tensor_tensor(
        ang.rearrange("p (t j) -> p t j", j=32),
        pos_sb[:, :, None].to_broadcast([P, NT, 32]),
        invf_sb[:, None, :].to_broadcast([P, NT, 32]),
        ALU.mult,
    )
    # round(ang / 2pi) via magic-number rounding
    kf = scr.tile([P, NT * 32], f32, tag=f"kf{tag}")
    nc.vector.tensor_scalar(kf, ang, float(INV2PI), float(MAGIC), ALU.mult, ALU.add)
    nc.vector.tensor_scalar(kf, kf, float(MAGIC), None, ALU.subtract)
    angr = scr.tile([P, NT * 32], f32, tag=f"angr{tag}")
    nc.vector.cody_waite_cascade(
        out=angr, x=ang, k=kf, c1=float(C1), c2=float(C2), c3=float(C3)
    )
    # cos argument: wrap(angr + pi/2) into [-pi, pi]
    nc.vector.add_range_wrap(
        out=kf, in_=angr, shift=math.pi / 2, bound=math.pi, period=2 * math.pi
    )
    sint = scr.tile([P, NT, 32], f32, tag=f"sin{tag}")
    cost = scr.tile([P, NT, 32], f32, tag=f"cos{tag}")
    nc.scalar.activation(sint.rearrange("p t j -> p (t j)"), angr, AF.Sin)
    nc.scalar.activation(cost.rearrange("p t j -> p (t j)"), kf, AF.Sin)

    tabs = []
    for name, trig, wlo in (
        ("cw1", cost, True),
        ("cw2", cost, False),
        ("sw1", sint, True),
        ("sw2", sint, False),
    ):
        t = pool.tile([P, NT, 32], bf16, tag=f"{name}{tag}")
        wsl = w_sb[:, 0:32] if wlo else w_sb[:, 32:64]
        nc.vector.tensor_tensor(
            t, trig, wsl[:, None, :].to_broadcast([P, NT, 32]), ALU.mult
        )
        tabs.append(t)
    return tabs


def _norm_rope(nc, tc, pools, xnat, rsq, tabs, tag):
    """xnat [P, NT, QD] f32 (+ rsq [P, NT, HPC] f32) -> roped, normalized bf16
    [P, NT, QD]."""
    from concourse import mybir

    bf16 = mybir.dt.bfloat16
    ALU = mybir.AluOpType
    acts, ascr = pools
    cw1, cw2, sw1, sw2 = tabs

    xb = acts.tile([P, NT, HPC, HD], bf16, tag="xb")
    x4 = xnat[:, :, :].rearrange("p t (h d) -> p t h d", h=HPC)
    nc.vector.tensor_tensor(
        xb, x4, rsq[:, :, :, None].to_broadcast([P, NT, HPC, HD]), ALU.mult
    )

    xhat = acts.tile([P, NT, HPC, 2, 32], bf16, tag="xhat")
    x1 = xb[:, :, :, 0:32]
    x2 = xb[:, :, :, 32:64]
    sh4 = [P, NT, HPC, 32]

    def bc(t):
        return t[:, :, None, :].to_broadcast(sh4)

    a = ascr.tile(sh4, bf16, tag="ra")
    b = ascr.tile(sh4, bf16, tag="rb")
    nc.vector.tensor_tensor(a, x1, bc(cw1), ALU.mult)
    nc.vector.tensor_tensor(b, x2, bc(sw2), ALU.mult)
    nc.vector.tensor_sub(xhat[:, :, :, 0, :], a, b)
    a2 = ascr.tile(sh4, bf16, tag="ra")
    b2 = ascr.tile(sh4, bf16, tag="rb")
    nc.vector.tensor_tensor(a2, x1, bc(sw1), ALU.mult)
    nc.vector.tensor_tensor(b2, x2, bc(cw2), ALU.mult)
    nc.vector.tensor_add(xhat[:, :, :, 1, :], a2, b2)
    return xhat


def _build():
    import concourse.bass as bass
    import concourse.tile as tile
    from concourse import bacc, mybir

    f32 = mybir.dt.float32
    bf16 = mybir.dt.bfloat16
    AF = mybir.ActivationFunctionType
    ALU = mybir.AluOpType

    nc = bacc.Bacc(
        "TRN2", target_bir_lowering=False, debug=False, num_devices=NCORES
    )

    tgt_t = nc.dram_tensor("tgt_t", [DIM, N], f32, kind="ExternalInput").ap()
    src_t = nc.dram_tensor("src_t", [DIM, N], f32, kind="ExternalInput").ap()
    wq_d = nc.dram_tensor("wq", [DIM, QD], f32, kind="ExternalInput").ap()
    wkv_d = nc.dram_tensor("wkv", [DIM, 2 * QD], f32, kind="ExternalInput").ap()
    wo_d = nc.dram_tensor("wo", [QD, DIM], f32, kind="ExternalInput").ap()
    tpos = nc.dram_tensor("tpos", [N], f32, kind="ExternalInput").ap()
    spos = nc.dram_tensor("spos", [N], f32, kind="ExternalInput").ap()
    qw_d = nc.dram_tensor("qw", [HD], f32, kind="ExternalInput").ap()
    kw_d = nc.dram_tensor("kw", [HD], f32, kind="ExternalInput").ap()
    out_d = nc.dram_tensor("out", [N, DIM], f32, kind="ExternalOutput").ap()

    invf_np = np.float32(ROPE_THETA) ** (
        -np.arange(0, HD, 2, dtype=np.float32) / np.float32(HD)
    )
    invf_dram = nc.inline_tensor(invf_np.astype(np.float32), "invf").ap()

    TWO_PI = 2 * math.pi
    C1 = np.float32(6.28125)
    C2 = np.float32(TWO_PI - float(C1))
    C3 = np.float32(TWO_PI - float(C1) - float(C2))
    MAGIC = np.float32(1.5 * 2**23)
    INV2PI = np.float32(1.0 / TWO_PI)
    consts = (INV2PI, MAGIC, C1, C2, C3)

    def bcast_ap(src, parts):
        return bass.AP(tensor=src.tensor, offset=src.offset, ap=[[0, parts]] + src.ap)

    DEBUG = bool(int(os.environ.get("KERNEL_DEBUG", "0")))
    dbg_done = set()

    def dbg(name, ap):
        if not DEBUG or name in dbg_done:
            return
        dbg_done.add(name)
        t = nc.dram_tensor(f"d_{name}", list(ap.shape), ap.dtype, kind="ExternalOutput").ap()
        nc.sync.dma_start(out=t, in_=ap)

    with tile.TileContext(nc) as tc:
        with (
            tc.tile_pool(name="persist", bufs=1) as persist,
            tc.tile_pool(name="dramp", bufs=1, space="DRAM") as dramp,
        ):
            # small broadcast loads
            qw_sb = persist.tile([P, HD], f32, tag="qw")
            kw_sb = persist.tile([P, HD], f32, tag="kw")
            invf_sb = persist.tile([P, 32], f32, tag="invf")
            nc.gpsimd.dma_start(out=qw_sb, in_=bcast_ap(qw_d, P))
            nc.gpsimd.dma_start(out=kw_sb, in_=bcast_ap(kw_d, P))
            nc.gpsimd.dma_start(out=invf_sb, in_=bcast_ap(invf_dram, P))

            eps_sb = persist.tile([P, 1], mybir.dt.float32, tag="eps")
            nc.vector.memset(eps_sb, EPS)
            lnq_sb = persist.tile([P, 1], mybir.dt.float32, tag="lnq")
            nc.vector.memset(lnq_sb, math.log(1.0 / 8.0))

            wo_bf = persist.tile([P, 2, DIM], bf16, tag="wo")
            for cc in range(2):
                nc.gpsimd.dma_start(
                    out=wo_bf[:, cc], in_=wo_d[cc * P : (cc + 1) * P, :]
                )

            with tc.tile_pool(name="trigscr", bufs=1) as trigscr:
                tabs_q = _build_trig(
                    nc, tc, persist, trigscr, tpos, qw_sb, invf_sb, consts, "q"
                )
                tabs_k = _build_trig(
                    nc, tc, persist, trigscr, spos, kw_sb, invf_sb, consts, "k"
                )

            kT = persist.tile([P, 2, N], bf16, tag="kT")
            qT = persist.tile([P, 2, N], bf16, tag="qT")
            vaug = persist.tile([P, NT, HPC, HD + 1], bf16, tag="vaug")
            nc.vector.memset(vaug, 1.0)
            oT = [persist.tile([P, N], bf16, tag=f"oT{i}", name=f"oT{i}") for i in range(2)]

            with (
                tc.tile_pool(name="acts", bufs=1) as acts,
                tc.tile_pool(name="ascr", bufs=2) as ascr,
                tc.tile_pool(name="ppsum", bufs=4, space="PSUM") as ppsum,
            ):
                # activation loads with f32->bf16 cast on the DMA (SWDGE)
                xs_bf = []
                xt_bf = []
                wkv_bf = []
                wq_bf = []
                for kc in range(KC):
                    t = acts.tile([P, N], bf16, tag=f"xs{kc}")
                    nc.gpsimd.dma_start(out=t, in_=src_t[kc * P : (kc + 1) * P, :])
                    xs_bf.append(t)
                    t = acts.tile([P, N], bf16, tag=f"xt{kc}")
                    nc.gpsimd.dma_start(out=t, in_=tgt_t[kc * P : (kc + 1) * P, :])
                    xt_bf.append(t)
                    t = acts.tile([P, 2 * QD], bf16, tag=f"wkv{kc}")
                    nc.gpsimd.dma_start(out=t, in_=wkv_d[kc * P : (kc + 1) * P, :])
                    wkv_bf.append(t)
                    t = acts.tile([P, QD], bf16, tag=f"wq{kc}")
                    nc.gpsimd.dma_start(out=t, in_=wq_d[kc * P : (kc + 1) * P, :])
                    wq_bf.append(t)

                # ---- KV projection ----
                knat = acts.tile([P, NT, QD], f32, tag="knat")
                for mc in range(NT):
                    ps = ppsum.tile([P, 2 * QD], f32, tag="kvps")
                    for kc in range(KC):
                        nc.tensor.matmul(
                            ps,
                            lhsT=xs_bf[kc][:, mc * P : (mc + 1) * P],
                            rhs=wkv_bf[kc],
                            start=(kc == 0),
                            stop=(kc == KC - 1),
                        )
                    nc.scalar.copy(knat[:, mc], ps[:, 0:QD])
                    nc.scalar.copy(
                        vaug[:, mc, :, 0:HD],
                        ps[:, QD : 2 * QD].rearrange("p (h d) -> p h d", h=HPC),
                    )

                # ---- K norm stats ----
                sq = acts.tile([P, NT, HPC, HD], f32, tag="sq")
                k4 = knat.rearrange("p t (h d) -> p t h d", h=HPC)
                nc.vector.tensor_tensor(sq, k4, k4, ALU.mult)
                kss = persist.tile([P, NT, HPC], f32, tag="kss")
                nc.vector.tensor_reduce(
                    kss, sq, axis=mybir.AxisListType.X, op=ALU.add
                )
                krsq = persist.tile([P, NT, HPC], f32, tag="krsq")
                nc.scalar.activation(kss, kss, AF.Ln, scale=1.0 / HD, bias=eps_sb)
                nc.scalar.activation(krsq, kss, AF.Exp, scale=-0.5)

                dbg("knat", knat)
                dbg("krsq", krsq)
                khat = _norm_rope(nc, tc, (acts, ascr), knat, krsq, tabs_k, "k")
                dbg("khat", khat)

                # ---- Q projection ----
                qnat = acts.tile([P, NT, QD], f32, tag="qnat")
                for mc in range(NT):
                    ps = ppsum.tile([P, QD], f32, tag="qps")
                    for kc in range(KC):
                        nc.tensor.matmul(
                            ps,
                            lhsT=xt_bf[kc][:, mc * P : (mc + 1) * P],
                            rhs=wq_bf[kc],
                            start=(kc == 0),
                            stop=(kc == KC - 1),
                        )
                    nc.scalar.copy(qnat[:, mc], ps)

                q4 = qnat.rearrange("p t (h d) -> p t h d", h=HPC)
                nc.vector.tensor_tensor(sq, q4, q4, ALU.mult)
                qss = persist.tile([P, NT, HPC], f32, tag="qss")
                nc.vector.tensor_reduce(
                    qss, sq, axis=mybir.AxisListType.X, op=ALU.add
                )
                qrsq = persist.tile([P, NT, HPC], f32, tag="qrsq")
                nc.scalar.activation(qss, qss, AF.Ln, scale=1.0 / HD, bias=eps_sb)
                # fold the 1/sqrt(hd)=1/8 score scale into q's rsq
                nc.scalar.activation(qrsq, qss, AF.Exp, scale=-0.5, bias=lnq_sb)

                dbg("qnat", qnat)
                dbg("qrsq", qrsq)
                qhat = _norm_rope(nc, tc, (acts, ascr), qnat, qrsq, tabs_q, "q")
                dbg("qhat", qhat)

                # ---- transpose K_hat, Q_hat via DRAM bounce + DMA xbar ----
                for xhat, xT, nm in ((khat, kT, "k"), (qhat, qT, "q")):
                    scr = dramp.tile([N, QD], bf16, tag=f"scr{nm}")
                    nc.sync.dma_start(
                        out=scr.rearrange("(c p) d -> p c d", p=P), in_=xhat
                    )
                    for half in range(2):
                        nc.sync.dma_start(
                            out=xT[:, half, :],
                            in_=scr[:, half * P : (half + 1) * P],
                            transpose=True,
                        )

            dbg("kT", kT)
            dbg("qT", qT)
            dbg("vaug", vaug)
            dbg("cw1q", tabs_q[0])
            dbg("sw1q", tabs_q[2])
            # ---- attention + output projection ----
            with (
                tc.tile_pool(name="ptp", bufs=2) as ptp,
                tc.tile_pool(name="dnp", bufs=4) as dnp,
                tc.tile_pool(name="ostage", bufs=4) as ostage,
                tc.tile_pool(name="spsum", bufs=1, space="PSUM") as spsum,
                tc.tile_pool(name="avpsum", bufs=1, space="PSUM") as avpsum,
                tc.tile_pool(name="wopsum", bufs=2, space="PSUM") as wopsum,
            ):
                for qb in range(QB):
                    for hp in range(2):
                        pt = ptp.tile([P, NT, 2, QW], bf16, tag="pt")
                        av = [
                            avpsum.tile([P, QW], f32, tag=f"av{i}", name=f"av{i}") for i in range(2)
                        ]
                        for g in range(NT // 2):
                            sp = spsum.tile([P, 2, 2, QW], f32, tag="sstage")
                            for ci in range(2):
                                mc = 2 * g + ci
                                for i in range(2):
                                    pp = slice(i * 64, (i + 1) * 64)
                                    nc.tensor.matmul(
                                        sp[:, ci, i, :],
                                        lhsT=kT[pp, hp, mc * P : (mc + 1) * P],
                                        rhs=qT[pp, hp, qb * QW : (qb + 1) * QW],
                                        start=True,
                                        stop=True,
                                        tile_position=(i * 64, 0),
                                    )
                            nc.scalar.activation(pt[:, 2 * g : 2 * g + 2], sp, AF.Exp)
                            for ci in range(2):
                                mc = 2 * g + ci
                                for i in range(2):
                                    nc.tensor.matmul(
                                        av[i][0 : HD + 1, :],
                                        lhsT=vaug[:, mc, 2 * hp + i, :],
                                        rhs=pt[:, mc, i, :],
                                        start=(mc == 0),
                                        stop=(mc == NT - 1),
                                    )
                        if qb == 0 and hp == 0:
                            dbg("pt", pt)
                        for i in range(2):
                            den = dnp.tile([1, QW], f32, tag="den")
                            nc.vector.tensor_copy(den, av[i][HD : HD + 1, :])
                            dn = dnp.tile([1, QW], f32, tag="dn")
                            nc.vector.reciprocal_approx_fast(out=dn, in_=den)
                            if qb == 0 and hp == 0 and i == 0:
                                dbg("den", den)
                                dbg("dn", dn)
                            dnd = dramp.tile([QW], f32, tag="dnd")
                            nc.sync.dma_start(out=dnd, in_=dn)
                            dnb = dnp.tile([HD, QW], f32, tag="dnb")
                            nc.sync.dma_start(out=dnb, in_=bcast_ap(dnd, HD))
                            if qb == 0 and hp == 0 and i == 0:
                                dbg("dnb", dnb)
                            nc.vector.tensor_tensor(
                                oT[hp][i * HD : (i + 1) * HD, qb * QW : (qb + 1) * QW],
                                av[i][0:HD, :],
                                dnb,
                                ALU.mult,
                            )
                    if qb == QB - 1:
                        dbg("oT0", oT[0])
                        dbg("oT1", oT[1])
                    # output projection for this q block
                    for ti in range(QW // P):
                        t0 = qb * QW + ti * P
                        ost = ostage.tile([P, DIM], f32, tag="ost")
                        for od in range(2):
                            wps = wopsum.tile([P, 512], f32, tag="wops")
                            for cc in range(2):
                                nc.tensor.matmul(
                                    wps,
                                    lhsT=oT[cc][:, t0 : t0 + P],
                                    rhs=wo_bf[:, cc, od * 512 : (od + 1) * 512],
                                    start=(cc == 0),
                                    stop=(cc == 1),
                                )
                            nc.vector.tensor_copy(ost[:, od * 512 : (od + 1) * 512], wps)
                        nc.sync.dma_start(out=out_d[t0 : t0 + P, :], in_=ost)

    nc.compile()
    return nc


def _get_nc():
    if "nc" not in _CACHE:
        _CACHE["nc"] = _build()
    return _CACHE["nc"]


def _shard(inputs):
    tgt = np.asarray(inputs["tgt"], np.float32)
    src = np.asarray(inputs["src"], np.float32)
    tgt_pos = np.asarray(inputs["tgt_pos"], np.float32)
    src_pos = np.asarray(inputs["src_pos"], np.float32)
    Wq = np.asarray(inputs["Wq"], np.float32)
    Wkv = np.asarray(inputs["Wkv"], np.float32)
    Wo = np.asarray(inputs["Wo"], np.float32)
    qw = np.asarray(inputs["q_norm_w"], np.float32)
    kw = np.asarray(inputs["k_norm_w"], np.float32)

    in_maps = []
    for c in range(NCORES):
        b, hg = divmod(c, 4)
        cs = slice(hg * QD, (hg + 1) * QD)
        in_maps.append(
            {
                "tgt_t": np.ascontiguousarray(tgt[b].T),
                "src_t": np.ascontiguousarray(src[b].T),
                "wq": np.ascontiguousarray(Wq[:, cs]),
                "wkv": np.ascontiguousarray(
                    np.concatenate([Wkv[:, cs], Wkv[:, DIM:][:, cs]], axis=1)
                ),
                "wo": np.ascontiguousarray(Wo[cs, :]),
                "tpos": np.ascontiguousarray(tgt_pos[b]),
                "spos": np.ascontiguousarray(src_pos[b]),
                "qw": np.ascontiguousarray(qw),
                "kw": np.ascontiguousarray(kw),
            }
        )
    return in_maps


def _install_ntff_shim():
    """Provide antenv.axon_hooks (missing in this image) so trace=True can
    capture NTFF profiles through libaxon_pjrt.so."""
    import sys
    import types
    import contextlib
    import ctypes

    if "antenv.axon_hooks" in sys.modules:
        return
    so_path = "/opt/axon/libaxon_pjrt.so"
    if not os.path.exists(so_path):
        return
    lib = ctypes.CDLL(so_path)
    if not hasattr(lib, "axon_start_nrt_profile"):
        return
    lib.axon_start_nrt_profile.argtypes = [
        ctypes.POINTER(ctypes.c_int64),
        ctypes.c_size_t,
    ]
    lib.axon_start_nrt_profile.restype = ctypes.c_int64
    lib.axon_stop_nrt_profile.argtypes = [ctypes.c_char_p]
    lib.axon_stop_nrt_profile.restype = ctypes.c_int64

    @contextlib.contextmanager
    def _hook(output_dir, device_ids):
        import jax

        jax.devices()
        if device_ids:
            ids = (ctypes.c_int64 * len(device_ids))(*device_ids)
            rc = lib.axon_start_nrt_profile(ids, len(device_ids))
        else:
            rc = lib.axon_start_nrt_profile(None, 0)
        if rc != 0:
            raise RuntimeError(f"axon_start_nrt_profile rc={rc}")
        try:
            yield
        finally:
            n = lib.axon_stop_nrt_profile(str(output_dir).encode())
            print(f"ntff profile: {n} file(s) written to {output_dir}")

    mod = types.ModuleType("antenv.axon_hooks")
    mod.get_axon_ntff_profile_hook = lambda: _hook
    mod.set_axon_ntff_profile_hook = lambda h: None
    sys.modules["antenv.axon_hooks"] = mod


def kernel(**inputs) -> np.ndarray:
    global LAST_RESULTS
    from concourse.bass_utils import run_bass_kernel_spmd

    nc = _get_nc()
    in_maps = _shard(inputs)
    trace = bool(int(os.environ.get("KERNEL_TRACE", "0")))
    if trace:
        _install_ntff_shim()
    res = run_bass_kernel_spmd(
        nc, in_maps, core_ids=list(range(NCORES)), trace=trace
    )
    LAST_RESULTS = res
    out = np.zeros((B, N, DIM), np.float32)
    for c in range(NCORES):
        out[c // 4] += res.results[c]["out"]
    return out


# revision 10
# speedup vs baseline: 1.3781x; 1.3781x over previous
"""Trainium2 Bass kernel for cross-attention (b=2, n=m=2048, dim=1024, 16 heads x 64)
with QK-RMSNorm and rotate-half RoPE (float positions), distributed over 8 NeuronCores.

Sharding: core c handles batch b = c//4 and head group hg = c%4 (4 heads each).
Wq/Wkv are column-sharded by head, Wo row-sharded; each core emits a partial
[2048, 1024] output which the host sums over the 4 cores of each batch (the
row-parallel all-reduce done at unshard time).

Device dataflow per core (all matmuls bf16, softmax f32 on ScalarE):
  tgt^T/src^T (host-transposed f32) --gpsimd DMA cast--> bf16 [128, 8, 2048]
  KV = src @ Wkv        (PE, natural layout [t, 512])
  Q  = tgt @ Wq         (PE, natural layout [t, 256])
  RMS stats (DVE square+reduce) -> rsq = exp(-0.5*ln(ms+eps)) (ACT, ln/exp set)
  normalize+cast (DVE), RoPE via w-folded sin/cos tables (DVE TT ops; trig from
  ACT Sin with Cody-Waite range reduction; cos = sin(x+pi/2))
  Q_hat/K_hat --DRAM bounce--> DMA-xbar transpose --> [hd, t] layout
  S^T = K_hat^T.T @ Q_hat^T   (row-tiled head pairs, K_c=64, full PE rate)
  P^T = exp(S^T)              (ACT, psum->sbuf bf16, [128, 1024] ops)
  O^T/denom = [V|1].T @ P^T   (PE, M=65: AV and softmax denominator together)
  normalize O^T (DVE, reciprocal + DMA partition-broadcast)
  out = O^T.T @ Wo            (PE, accumulate over c-chunks, natural layout)
"""

import math
import os

import numpy as np

B, N, DIM, H, HD = 2, 2048, 1024, 16, 64
NCORES = 8
HPC = 4  # heads per core
QD = HPC * HD  # 256
P = 128
NT = N // P  # 16 token chunks
KC = DIM // P  # 8 contraction chunks
QB = 4  # q blocks of 512
QW = N // QB  # 512
ROPE_THETA = 10000.0
EPS = float(np.finfo(np.float32).eps)

_CACHE = {}
LAST_RESULTS = None


def _build_trig(nc, tc, pool, scr, pos_dram, w_sb, invf_sb, consts, tag):
    """sin/cos tables with RMSNorm-weight w folded in. Returns (cw1, cw2, sw1, sw2),
    each [P, NT, 32] bf16: cw1=cos*w[0:32], cw2=cos*w[32:64], sw1=sin*w[0:32],
    sw2=sin*w[32:64]."""
    import concourse.bass as bass
    from concourse import mybir

    f32 = mybir.dt.float32
    bf16 = mybir.dt.bfloat16
    AF = mybir.ActivationFunctionType
    ALU = mybir.AluOpType
    INV2PI, MAGIC, C1, C2, C3 = consts

    pos_sb = scr.tile([P, NT], f32, tag=f"pos{tag}")
    nc.sync.dma_start(out=pos_sb, in_=pos_dram.rearrange("(c p) -> p c", p=P))
    ang = scr.tile([P, NT * 32], f32, tag=f"ang{tag}")
    nc.vector.tensor_tensor(
        ang.rearrange("p (t j) -> p t j", j=32),
        pos_sb[:, :, None].to_broadcast([P, NT, 32]),
        invf_sb[:, None, :].to_broadcast([P, NT, 32]),
        ALU.mult,
    )
    # round(ang / 2pi) via magic-number rounding
    kf = scr.tile([P, NT * 32], f32, tag=f"kf{tag}")
    nc.vector.tensor_scalar(kf, ang, float(INV2PI), float(MAGIC), ALU.mult, ALU.add)
    nc.vector.tensor_scalar(kf, kf, float(MAGIC), None, ALU.subtract)
    angr = scr.tile([P, NT * 32], f32, tag=f"angr{tag}")
    nc.vector.cody_waite_cascade(
        out=angr, x=ang, k=kf, c1=float(C1), c2=float(C2), c3=float(C3)
    )
    # cos argument: wrap(angr + pi/2) into [-pi, pi]
    nc.vector.add_range_wrap(
        out=kf, in_=angr, shift=math.pi / 2, bound=math.pi, period=2 * math.pi
    )
    sint = scr.tile([P, NT, 32], f32, tag=f"sin{tag}")
    cost = scr.tile([P, NT, 32], f32, tag=f"cos{tag}")
    nc.scalar.activation(sint.rearrange("p t j -> p (t j)"), angr, AF.Sin)
    nc.scalar.activation(cost.rearrange("p t j -> p (t j)"), kf, AF.Sin)

    tabs = []
    for name, trig, wlo in (
        ("cw1", cost, True),
        ("cw2", cost, False),
        ("sw1", sint, True),
        ("sw2", sint, False),
    ):
        t = pool.tile([P, NT, 32], bf16, tag=f"{name}{tag}")
        wsl = w_sb[:, 0:32] if wlo else w_sb[:, 32:64]
        nc.vector.tensor_tensor(
            t, trig, wsl[:, None, :].to_broadcast([P, NT, 32]), ALU.mult
        )
        tabs.append(t)
    return tabs


def _norm_rope(nc, tc, pools, xnat, rsq, tabs, tag):
    """xnat [P, NT, QD] f32 (+ rsq [P, NT, HPC] f32) -> roped, normalized bf16
    [P, NT, QD]."""
    from concourse import mybir

    bf16 = mybir.dt.bfloat16
    ALU = mybir.AluOpType
    acts, ascr = pools
    cw1, cw2, sw1, sw2 = tabs

    xb = acts.tile([P, NT, HPC, HD], bf16, tag="xb")
    x4 = xnat[:, :, :].rearrange("p t (h d) -> p t h d", h=HPC)
    nc.vector.tensor_tensor(
        xb, x4, rsq[:, :, :, None].to_broadcast([P, NT, HPC, HD]), ALU.mult
    )

    xhat = acts.tile([P, NT, HPC, 2, 32], bf16, tag="xhat")
    x1 = xb[:, :, :, 0:32]
    x2 = xb[:, :, :, 32:64]
    sh4 = [P, NT, HPC, 32]

    def bc(t):
        return t[:, :, None, :].to_broadcast(sh4)

    a = ascr.tile(sh4, bf16, tag="ra")
    b = ascr.tile(sh4, bf16, tag="rb")
    nc.vector.tensor_tensor(a, x1, bc(cw1), ALU.mult)
    nc.vector.tensor_tensor(b, x2, bc(sw2), ALU.mult)
    nc.vector.tensor_sub(xhat[:, :, :, 0, :], a, b)
    a2 = ascr.tile(sh4, bf16, tag="ra")
    b2 = ascr.tile(sh4, bf16, tag="rb")
    nc.vector.tensor_tensor(a2, x1, bc(sw1), ALU.mult)
    nc.vector.tensor_tensor(b2, x2, bc(cw2), ALU.mult)
    nc.vector.tensor_add(xhat[:, :, :, 1, :], a2, b2)
    return xhat


def _build():
    import concourse.bass as bass
    import concourse.tile as tile
    from concourse import bacc, mybir

    f32 = mybir.dt.float32
    bf16 = mybir.dt.bfloat16
    AF = mybir.ActivationFunctionType
    ALU = mybir.AluOpType

    nc = bacc.Bacc(
        "TRN2", target_bir_lowering=False, debug=False, num_devices=NCORES
    )

    tgt_t = nc.dram_tensor("tgt_t", [DIM, N], f32, kind="ExternalInput").ap()
    src_t = nc.dram_tensor("src_t", [DIM, N], f32, kind="ExternalInput").ap()
    wq_d = nc.dram_tensor("wq", [DIM, QD], f32, kind="ExternalInput").ap()
    wkv_d = nc.dram_tensor("wkv", [DIM, 2 * QD], f32, kind="ExternalInput").ap()
    wo_d = nc.dram_tensor("wo", [QD, DIM], f32, kind="ExternalInput").ap()
    tpos = nc.dram_tensor("tpos", [N], f32, kind="ExternalInput").ap()
    spos = nc.dram_tensor("spos", [N], f32, kind="ExternalInput").ap()
    qw_d = nc.dram_tensor("qw", [HD], f32, kind="ExternalInput").ap()
    kw_d = nc.dram_tensor("kw", [HD], f32, kind="ExternalInput").ap()
    out_d = nc.dram_tensor("out", [N, DIM], f32, kind="ExternalOutput").ap()

    invf_np = np.float32(ROPE_THETA) ** (
        -np.arange(0, HD, 2, dtype=np.float32) / np.float32(HD)
    )
    invf_dram = nc.inline_tensor(invf_np.astype(np.float32), "invf").ap()

    TWO_PI = 2 * math.pi
    C1 = np.float32(6.28125)
    C2 = np.float32(TWO_PI - float(C1))
    C3 = np.float32(TWO_PI - float(C1) - float(C2))
    MAGIC = np.float32(1.5 * 2**23)
    INV2PI = np.float32(1.0 / TWO_PI)
    consts = (INV2PI, MAGIC, C1, C2, C3)

    def bcast_ap(src, parts):
        return bass.AP(tensor=src.tensor, offset=src.offset, ap=[[0, parts]] + src.ap)

    DEBUG = bool(int(os.environ.get("KERNEL_DEBUG", "0")))
    dbg_done = set()

    def dbg(name, ap):
        if not DEBUG or name in dbg_done:
            return
        dbg_done.add(name)
        t = nc.dram_tensor(f"d_{name}", list(ap.shape), ap.dtype, kind="ExternalOutput").ap()
        nc.sync.dma_start(out=t, in_=ap)

    with tile.TileContext(nc) as tc:
        with (
            tc.tile_pool(name="persist", bufs=1) as persist,
            tc.tile_pool(name="dramp", bufs=1, space="DRAM") as dramp,
        ):
            # small broadcast loads
            qw_sb = persist.tile([P, HD], f32, tag="qw")
            kw_sb = persist.tile([P, HD], f32, tag="kw")
            invf_sb = persist.tile([P, 32], f32, tag="invf")
            nc.gpsimd.dma_start(out=qw_sb, in_=bcast_ap(qw_d, P))
            nc.gpsimd.dma_start(out=kw_sb, in_=bcast_ap(kw_d, P))
            nc.gpsimd.dma_start(out=invf_sb, in_=bcast_ap(invf_dram, P))

            eps_sb = persist.tile([P, 1], mybir.dt.float32, tag="eps")
            nc.vector.memset(eps_sb, EPS)
            lnq_sb = persist.tile([P, 1], mybir.dt.float32, tag="lnq")
            nc.vector.memset(lnq_sb, math.log(1.0 / 8.0))

            wo_bf = persist.tile([P, 2, DIM], bf16, tag="wo")
            for cc in range(2):
                nc.gpsimd.dma_start(
                    out=wo_bf[:, cc], in_=wo_d[cc * P : (cc + 1) * P, :]
                )

            with tc.tile_pool(name="trigscr", bufs=1) as trigscr:
                tabs_q = _build_trig(
                    nc, tc, persist, trigscr, tpos, qw_sb, invf_sb, consts, "q"
                )
                tabs_k = _build_trig(
                    nc, tc, persist, trigscr, spos, kw_sb, invf_sb, consts, "k"
                )

            kT = persist.tile([P, 2, N], bf16, tag="kT")
            qT = persist.tile([P, 2, N], bf16, tag="qT")
            vaug = persist.tile([P, NT, HPC, HD + 1], bf16, tag="vaug")
            nc.vector.memset(vaug, 1.0)
            oT = [persist.tile([P, N], bf16, tag=f"oT{i}", name=f"oT{i}") for i in range(2)]

            with (
                tc.tile_pool(name="acts", bufs=1) as acts,
                tc.tile_pool(name="ascr", bufs=2) as ascr,
                tc.tile_pool(name="ppsum", bufs=4, space="PSUM") as ppsum,
            ):
                # activation loads with f32->bf16 cast on the DMA (SWDGE)
                xs_bf = []
                xt_bf = []
                wkv_bf = []
                wq_bf = []
                for kc in range(KC):
                    t = acts.tile([P, N], bf16, tag=f"xs{kc}")
                    nc.gpsimd.dma_start(out=t, in_=src_t[kc * P : (kc + 1) * P, :])
                    xs_bf.append(t)
                    t = acts.tile([P, N], bf16, tag=f"xt{kc}")
                    nc.gpsimd.dma_start(out=t, in_=tgt_t[kc * P : (kc + 1) * P, :])
                    xt_bf.append(t)
                    t = acts.tile([P, 2 * QD], bf16, tag=f"wkv{kc}")
                    nc.gpsimd.dma_start(out=t, in_=wkv_d[kc * P : (kc + 1) * P, :])
                    wkv_bf.append(t)
                    t = acts.tile([P, QD], bf16, tag=f"wq{kc}")
                    nc.gpsimd.dma_start(out=t, in_=wq_d[kc * P : (kc + 1) * P, :])
                    wq_bf.append(t)

                # ---- KV projection ----
                knat = acts.tile([P, NT, QD], f32, tag="knat")
                for mc in range(NT):
                    ps = ppsum.tile([P, 2 * QD], f32, tag="kvps")
                    for kc in range(KC):
                        nc.tensor.matmul(
                            ps,
                            lhsT=xs_bf[kc][:, mc * P : (mc + 1) * P],
                            rhs=wkv_bf[kc],
                            start=(kc == 0),
                            stop=(kc == KC - 1),
                        )
                    nc.scalar.copy(knat[:, mc], ps[:, 0:QD])
                    nc.scalar.copy(
                        vaug[:, mc, :, 0:HD],
                        ps[:, QD : 2 * QD].rearrange("p (h d) -> p h d", h=HPC),
                    )

                # ---- K norm stats ----
                sq = acts.tile([P, NT, HPC, HD], f32, tag="sq")
                k4 = knat.rearrange("p t (h d) -> p t h d", h=HPC)
                nc.vector.tensor_tensor(sq, k4, k4, ALU.mult)
                kss = persist.tile([P, NT, HPC], f32, tag="kss")
                nc.vector.tensor_reduce(
                    kss, sq, axis=mybir.AxisListType.X, op=ALU.add
                )
                krsq = persist.tile([P, NT, HPC], f32, tag="krsq")
                nc.scalar.activation(kss, kss, AF.Ln, scale=1.0 / HD, bias=eps_sb)
                nc.scalar.activation(krsq, kss, AF.Exp, scale=-0.5)

                dbg("knat", knat)
                dbg("krsq", krsq)
                khat = _norm_rope(nc, tc, (acts, ascr), knat, krsq, tabs_k, "k")
                dbg("khat", khat)

                # ---- Q projection ----
                qnat = acts.tile([P, NT, QD], f32, tag="qnat")
                for mc in range(NT):
                    ps = ppsum.tile([P, QD], f32, tag="qps")
                    for kc in range(KC):
                        nc.tensor.matmul(
                            ps,
                            lhsT=xt_bf[kc][:, mc * P : (mc + 1) * P],
                            rhs=wq_bf[kc],
                            start=(kc == 0),
                            stop=(kc == KC - 1),
                        )
                    nc.scalar.copy(qnat[:, mc], ps)

                q4 = qnat.rearrange("p t (h d) -> p t h d", h=HPC)
                nc.vector.tensor_tensor(sq, q4, q4, ALU.mult)
                qss = persist.tile([P, NT, HPC], f32, tag="qss")
                nc.vector.tensor_reduce(
                    qss, sq, axis=mybir.AxisListType.X, op=ALU.add
                )
                qrsq = persist.tile([P, NT, HPC], f32, tag="qrsq")
                nc.scalar.activation(qss, qss, AF.Ln, scale=1.0 / HD, bias=eps_sb)
                # fold the 1/sqrt(hd)=1/8 score scale into q's rsq
                nc.scalar.activation(qrsq, qss, AF.Exp, scale=-0.5, bias=lnq_sb)

                dbg("qnat", qnat)
                dbg("qrsq", qrsq)
                qhat = _norm_rope(nc, tc, (acts, ascr), qnat, qrsq, tabs_q, "q")
                dbg("qhat", qhat)

                # ---- transpose K_hat, Q_hat via DRAM bounce + DMA xbar ----
                for xhat, xT, nm in ((khat, kT, "k"), (qhat, qT, "q")):
                    scr = dramp.tile([N, QD], bf16, tag=f"scr{nm}")
                    nc.sync.dma_start(
                        out=scr.rearrange("(c p) d -> p c d", p=P), in_=xhat
                    )
                    for half in range(2):
                        nc.sync.dma_start(
                            out=xT[:, half, :],
                            in_=scr[:, half * P : (half + 1) * P],
                            transpose=True,
                        )

            dbg("kT", kT)
            dbg("qT", qT)
            dbg("vaug", vaug)
            dbg("cw1q", tabs_q[0])
            dbg("sw1q", tabs_q[2])
            # ---- attention + output projection ----
            with (
                tc.tile_pool(name="ptp", bufs=2) as ptp,
                tc.tile_pool(name="dnp", bufs=4) as dnp,
                tc.tile_pool(name="ostage", bufs=4) as ostage,
                tc.tile_pool(name="spsum", bufs=2, space="PSUM") as spsum,
                tc.tile_pool(name="avpsum", bufs=1, space="PSUM") as avpsum,
                tc.tile_pool(name="wopsum", bufs=2, space="PSUM") as wopsum,
            ):
                for qb in range(QB):
                    for hp in range(2):
                        pt = ptp.tile([P, NT, 2, QW], bf16, tag="pt")
                        av = [
                            avpsum.tile([P, QW], f32, tag=f"av{i}", name=f"av{i}") for i in range(2)
                        ]
                        for mc in range(NT):
                            sp = spsum.tile([P, 2, QW], f32, tag="sstage")
                            for i in range(2):
                                pp = slice(i * 64, (i + 1) * 64)
                                nc.tensor.matmul(
                                    sp[:, i, :],
                                    lhsT=kT[pp, hp, mc * P : (mc + 1) * P],
                                    rhs=qT[pp, hp, qb * QW : (qb + 1) * QW],
                                    start=True,
                                    stop=True,
                                    tile_position=(i * 64, 0),
                                )
                            nc.scalar.activation(pt[:, mc], sp, AF.Exp)
                            for i in range(2):
                                nc.tensor.matmul(
                                    av[i][0 : HD + 1, :],
                                    lhsT=vaug[:, mc, 2 * hp + i, :],
                                    rhs=pt[:, mc, i, :],
                                    start=(mc == 0),
                                    stop=(mc == NT - 1),
                                )
                        if qb == 0 and hp == 0:
                            dbg("pt", pt)
                        for i in range(2):
                            den = dnp.tile([1, QW], f32, tag="den")
                            nc.vector.tensor_copy(den, av[i][HD : HD + 1, :])
                            dn = dnp.tile([1, QW], f32, tag="dn")
                            nc.vector.reciprocal_approx_fast(out=dn, in_=den)
                            if qb == 0 and hp == 0 and i == 0:
                                dbg("den", den)
                                dbg("dn", dn)
                            dnd = dramp.tile([QW], f32, tag="dnd")
                            nc.sync.dma_start(out=dnd, in_=dn)
                            dnb = dnp.tile([HD, QW], f32, tag="dnb")
                            nc.sync.dma_start(out=dnb, in_=bcast_ap(dnd, HD))
                            if qb == 0 and hp == 0 and i == 0:
                                dbg("dnb", dnb)
                            nc.vector.tensor_tensor(
                                oT[hp][i * HD : (i + 1) * HD, qb * QW : (qb + 1) * QW],
                                av[i][0:HD, :],
                                dnb,
                                ALU.mult,
                            )
                    if qb == QB - 1:
                        dbg("oT0", oT[0])
                        dbg("oT1", oT[1])
                    # output projection for this q block
                    for ti in range(QW // P):
                        t0 = qb * QW + ti * P
                        ost = ostage.tile([P, DIM], f32, tag="ost")
                        for od in range(2):
                            wps = wopsum.tile([P, 512], f32, tag="wops")
                            for cc in range(2):
                                nc.tensor.matmul(
                                    wps,
                                    lhsT=oT[cc][:, t0 : t0 + P],
                                    rhs=wo_bf[:, cc, od * 512 : (od + 1) * 512],
                                    start=(cc == 0),
                                    stop=(cc == 1),
                                )
                            nc.vector.tensor_copy(ost[:, od * 512 : (od + 1) * 512], wps)
                        nc.sync.dma_start(out=out_d[t0 : t0 + P, :], in_=ost)

    nc.compile()
    return nc


def _get_nc():
    if "nc" not in _CACHE:
        _CACHE["nc"] = _build()
    return _CACHE["nc"]


def _shard(inputs):
    tgt = np.asarray(inputs["tgt"], np.float32)
    src = np.asarray(inputs["src"], np.float32)
    tgt_pos = np.asarray(inputs["tgt_pos"], np.float32)
    src_pos = np.asarray(inputs["src_pos"], np.float32)
    Wq = np.asarray(inputs["Wq"], np.float32)
    Wkv = np.asarray(inputs["Wkv"], np.float32)
    Wo = np.asarray(inputs["Wo"], np.float32)
    qw = np.asarray(inputs["q_norm_w"], np.float32)
    kw = np.asarray(inputs["k_norm_w"], np.float32)

    in_maps = []
    for c in range(NCORES):
        b, hg = divmod(c, 4)
        cs = slice(hg * QD, (hg + 1) * QD)
        in_maps.append(
            {
                "tgt_t": np.ascontiguousarray(tgt[b].T),
                "src_t": np.ascontiguousarray(src[b].T),
                "wq": np.ascontiguousarray(Wq[:, cs]),
                "wkv": np.ascontiguousarray(
                    np.concatenate([Wkv[:, cs], Wkv[:, DIM:][:, cs]], axis=1)
                ),
                "wo": np.ascontiguousarray(Wo[cs, :]),
                "tpos": np.ascontiguousarray(tgt_pos[b]),
                "spos": np.ascontiguousarray(src_pos[b]),
                "qw": np.ascontiguousarray(qw),
                "kw": np.ascontiguousarray(kw),
            }
        )
    return in_maps


def _install_ntff_shim():
    """Provide antenv.axon_hooks (missing in this image) so trace=True can
    capture NTFF profiles through libaxon_pjrt.so."""
    import sys
    import types
    import contextlib
    import ctypes

    if "antenv.axon_hooks" in sys.modules:
        return
    so_path = "/opt/axon/libaxon_pjrt.so"
    if not os.path.exists(so_path):
        return
    lib = ctypes.CDLL(so_path)
    if not hasattr(lib, "axon_start_nrt_profile"):
        return
    lib.axon_start_nrt_profile.argtypes = [
        ctypes.POINTER(ctypes.c_int64),
        ctypes.c_size_t,
    ]
    lib.axon_start_nrt_profile.restype = ctypes.c_int64
    lib.axon_stop_nrt_profile.argtypes = [ctypes.c_char_p]
    lib.axon_stop_nrt_profile.restype = ctypes.c_int64

    @contextlib.contextmanager
    def _hook(output_dir, device_ids):
        import jax

        jax.devices()
        if device_ids:
            ids = (ctypes.c_int64 * len(device_ids))(*device_ids)
            rc = lib.axon_start_nrt_profile(ids, len(device_ids))
        else:
            rc = lib.axon_start_nrt_profile(None, 0)
        if rc != 0:
            raise RuntimeError(f"axon_start_nrt_profile rc={rc}")
        try:
            yield
        finally:
            n = lib.axon_stop_nrt_profile(str(output_dir).encode())
            print(f"ntff profile: {n} file(s) written to {output_dir}")

    mod = types.ModuleType("antenv.axon_hooks")
    mod.get_axon_ntff_profile_hook = lambda: _hook
    mod.set_axon_ntff_profile_hook = lambda h: None
    sys.modules["antenv.axon_hooks"] = mod


def kernel(**inputs) -> np.ndarray:
    global LAST_RESULTS
    from concourse.bass_utils import run_bass_kernel_spmd

    nc = _get_nc()
    in_maps = _shard(inputs)
    trace = bool(int(os.environ.get("KERNEL_TRACE", "0")))
    if trace:
        _install_ntff_shim()
    res = run_bass_kernel_spmd(
        nc, in_maps, core_ids=list(range(NCORES)), trace=trace
    )
    LAST_RESULTS = res
    out = np.zeros((B, N, DIM), np.float32)
    for c in range(NCORES):
        out[c // 4] += res.results[c]["out"]
    return out
